# revision 1
# baseline (speedup 1.0000x reference)
"""Trainium2 Bass kernel for nn_AdjGen (GNN message passing / adjacency generation).

Reference computation (N=4096 nodes, F=E=256, H=4 heads, hd=64):
    q = X @ Wq.T ; k = X @ Wk.T ; v = A @ Wv.T          (per-head reshapes)
    scores = (q . k) / sqrt(hd), diagonal masked to -inf
    attn   = softmax(scores, axis=keys)
    ctx    = attn @ v ; out = ctx @ Wo.T
    pred   = out @ Wd.T + bd
    result = A * sigmoid(pred)

Sharding: the N=4096 query dimension is split across 8 NeuronCores (512
queries each).  A is row-sharded to match; each core computes its 512 rows
of v = A @ Wv.T and an AllGather collective assembles the full v.  X and
all weights are replicated.

Everything on device runs in a "transposed" layout (queries along the SBUF
free dimension) so that every matmul contracts along partitions and the
softmax denominator comes out of the attention matmul itself via a
ones-column appended to v.  The diagonal (j == i) exclusion is done by
*including* the diagonal in the attention pass and then subtracting an
exactly recomputed correction term — this keeps the program identical on
every core (pure SPMD) with all core differences carried by input data.

Host-side staging: inputs are pre-transposed with numpy (A.T column shards,
X.T, W*.T) and the device writes the output transposed; the host transposes
back.  Only device (HW) execution time is the optimization target.
"""

import os

import numpy as np

N = 4096
F = 256
E = 256
H = 4
HD = 64
NCORES = 8
NS = N // NCORES  # 512 queries per core
CH = N // 128  # 32 key chunks of 128
SCALE = 1.0 / np.sqrt(HD)

# matmul dtype knob: "f32" (exact, 4 cyc/row) or "f32r" (full speed, reduced
# internal precision).  Applied to the large matmuls only.
MM_MODE = os.environ.get("KERNEL_MM_MODE", "f32r")

_cache = {}


def _build(mm_mode, n_devices=NCORES, use_collective=True):
    import concourse.bass as bass
    import concourse.mybir as mybir
    import concourse.tile as tile
    from concourse import bacc

    dt = mybir.dt
    f32 = dt.float32

    # float32r: 4-byte storage, PE rounds operands (~13-bit mantissa) on read
    # and streams at full rate (1 row/cycle at N>=256).  Tiles feeding
    # matmuls are declared float32r; DMA sources are bitcast so the verifier
    # sees a float32r producer chain.  Non-matmul readers bitcast back to
    # float32 (the raw bits are untouched).
    fr = dt.float32r if mm_mode == "f32r" else f32

    def rbc(ap):
        """Bitcast a float32 DRAM source AP for a float32r-tile DMA."""
        return ap.bitcast(fr) if mm_mode == "f32r" else ap

    nc = bacc.Bacc("TRN2", target_bir_lowering=False, debug=False,
                   num_devices=n_devices)

    # ---- kernel I/O (per-core DRAM tensors) ----
    at_d = nc.dram_tensor("at", [N, NS], f32, kind="ExternalInput")      # A[shard].T
    xq_d = nc.dram_tensor("xq", [F, NS], f32, kind="ExternalInput")      # X[shard].T
    wqt_d = nc.dram_tensor("wqt", [F, E], f32, kind="ExternalInput")     # Wq.T
    wkt_d = nc.dram_tensor("wkt", [F, E], f32, kind="ExternalInput")     # Wk.T
    wvt_d = nc.dram_tensor("wvt", [N, E], f32, kind="ExternalInput")     # Wv.T
    wot_d = nc.dram_tensor("wot", [E, E], f32, kind="ExternalInput")     # Wo.T
    wdt_d = nc.dram_tensor("wdt", [E, N], f32, kind="ExternalInput")     # Wd.T
    bd_d = nc.dram_tensor("bd", [CH, 128], f32, kind="ExternalInput")    # bias rows
    idn_d = nc.dram_tensor("idn", [128, 128], f32, kind="ExternalInput") # identity
    out_d = nc.dram_tensor("outt", [N, NS], f32, kind="ExternalOutput")  # result[shard].T

    at = at_d.ap()
    out = out_d.ap()

    with tile.TileContext(nc) as tc:
        from contextlib import ExitStack

        es = ExitStack()
        with es:
            res = es.enter_context(tc.tile_pool(name="res", bufs=1))
            strm = es.enter_context(tc.tile_pool(name="strm", bufs=4))
            wvp = es.enter_context(tc.tile_pool(name="wvp", bufs=4))
            expp = es.enter_context(tc.tile_pool(name="expp", bufs=4))
            workp = es.enter_context(tc.tile_pool(name="workp", bufs=2))
            # PSUM budget (8 banks): ps2 slots 3x[128,1024] = 6, ctx 2
            psum = es.enter_context(tc.tile_pool(name="psum", bufs=3, space="PSUM"))
            pctx = es.enter_context(tc.tile_pool(name="pctx", bufs=2, space="PSUM"))
            dram = es.enter_context(tc.tile_pool(name="dram", bufs=1, space="DRAM"))

            Exp = mybir.ActivationFunctionType.Exp
            Sig = mybir.ActivationFunctionType.Sigmoid
            EW = H * (HD + 1)          # per-head v width incl. ones column
            VB = NS * EW               # floats of v in the bounce block
            BB = VB + E * NS           # bounce block floats per rank (v + kT_own)

            # ---------- small resident loads ----------
            xq_t = []
            wqt_t = []
            wkt_t = []
            wot_t = []
            for fc in range(2):
                t = res.tile([128, NS], fr, name=f"xq{fc}", tag=f"xq{fc}")
                nc.sync.dma_start(t[:], rbc(xq_d.ap()[fc * 128:(fc + 1) * 128, :]))
                xq_t.append(t)
                t = res.tile([128, E], fr, name=f"wqt{fc}", tag=f"wqt{fc}")
                nc.sync.dma_start(t[:], rbc(wqt_d.ap()[fc * 128:(fc + 1) * 128, :]))
                wqt_t.append(t)
                t = res.tile([128, E], fr, name=f"wkt{fc}", tag=f"wkt{fc}")
                nc.sync.dma_start(t[:], rbc(wkt_d.ap()[fc * 128:(fc + 1) * 128, :]))
                wkt_t.append(t)
                t = res.tile([128, E], fr, name=f"wot{fc}", tag=f"wot{fc}")
                nc.sync.dma_start(t[:], rbc(wot_d.ap()[fc * 128:(fc + 1) * 128, :]))
                wot_t.append(t)
            bd_sb = res.tile([128, CH], f32, name="bd_sb", tag="bd")
            nc.sync.dma_start(bd_sb[:], bd_d.ap().rearrange("c p -> p c"))
            idn_sb = res.tile([128, 128], f32, name="idn_sb", tag="idn")
            nc.sync.dma_start(idn_sb[:], idn_d.ap()[:])
            ones_sb = res.tile([128, 1], f32, name="ones_sb", tag="ones")
            nc.vector.memset(ones_sb[:], 1.0)
            ones2 = res.tile([128, 64], f32, name="ones2", tag="ones2")
            nc.vector.memset(ones2[:], 1.0)
            ones4 = res.tile([128, H], f32, name="ones4", tag="ones4")
            nc.vector.memset(ones4[:], 1.0)

            bounce = dram.tile([BB], f32, name="bounce")
            vcb3 = bounce[0:VB].rearrange("(n h d) -> n h d", n=NS, h=H)
            kob = bounce[VB:BB].rearrange("(e q) -> e q", e=E)

            # ---------- qT and kT_own from the local X shard ----------
            qt_t = []
            kto_t = []
            for ec in range(2):
                ps = psum.tile([128, NS], f32, name="psq", tag="ps2")
                for fc in range(2):
                    nc.tensor.matmul(
                        ps[:],
                        wqt_t[fc][:, ec * 128:(ec + 1) * 128],
                        xq_t[fc][:],
                        start=(fc == 0), stop=(fc == 1),
                    )
                t = res.tile([128, NS], fr, name=f"qt{ec}", tag=f"qt{ec}")
                nc.vector.tensor_copy(t[:], ps[:])
                qt_t.append(t)

                ps2 = psum.tile([128, NS], f32, name="psko", tag="ps2")
                for fc in range(2):
                    nc.tensor.matmul(
                        ps2[:],
                        wkt_t[fc][:, ec * 128:(ec + 1) * 128],
                        xq_t[fc][:],
                        start=(fc == 0), stop=(fc == 1),
                    )
                t = res.tile([128, NS], fr, name=f"kto{ec}", tag=f"kto{ec}")
                nc.vector.tensor_copy(t[:], ps2[:])
                kto_t.append(t)
                nc.gpsimd.dma_start(
                    kob[ec * 128:(ec + 1) * 128, :], t.bitcast(f32)[:])

            # ---------- phase 1: v_c = A[shard] @ Wv.T, streamed over j ----------
            # at[j] pairs are fetched as one 512 KiB DMA to halve DMA count
            at_t = []
            # four independent accumulation banks borrowed from the pvc and
            # pctx pools (both otherwise idle during phase 1)
            ps_v = [
                psum.tile([128, E], f32, name="psva", tag="ps2"),
                psum.tile([128, E], f32, name="psvb", tag="ps2"),
                pctx.tile([128, E], f32, name="psvc", tag="ctx"),
                pctx.tile([128, E], f32, name="psvd", tag="ctx"),
            ]
            for j4 in range(CH // 4):
                t = res.tile([128, 4 * NS], fr, name=f"at{j4}", tag=f"at{j4}")
                nc.sync.dma_start(
                    t.rearrange("p (a q) -> p a q", a=4),
                    rbc(at[j4 * 512:(j4 + 1) * 512, :]).rearrange(
                        "(a p) q -> p a q", a=4),
                )
                at_t.append(t)
                wv_t = wvp.tile([128, 4 * E], fr, name=f"wv{j4}", tag="wv",
                                bufs=2)
                nc.sync.dma_start(
                    wv_t.rearrange("p (a e) -> p a e", a=4),
                    rbc(wvt_d.ap()[j4 * 512:(j4 + 1) * 512, :]).rearrange(
                        "(a p) e -> p a e", a=4),
                )
                for a in range(4):
                    j = 4 * j4 + a
                    for mc in range(4):
                        nc.tensor.matmul(
                            ps_v[mc][:],
                            at_t[j4][:, a * NS + mc * 128:a * NS + (mc + 1) * 128],
                            wv_t[:, a * E:(a + 1) * E],
                            start=(j == 0), stop=(j == CH - 1),
                        )
            vc_sb = []
            for mc in range(4):
                t = res.tile([128, E], f32, name=f"vc{mc}", tag=f"vc{mc}")
                nc.vector.tensor_copy(t[:], ps_v[mc][:])
                vc_sb.append(t)
                nc.gpsimd.dma_start(
                    vcb3[mc * 128:(mc + 1) * 128, :, 0:HD],
                    t.rearrange("p (h d) -> p h d", h=H),
                )
                nc.gpsimd.dma_start(
                    vcb3[mc * 128:(mc + 1) * 128, :, HD:HD + 1],
                    ones4.unsqueeze(-1),
                )

            # ---------- phase 2: one AllGather moves v (+ones) and kT ----------
            gb = dram.tile([NCORES * BB], f32, name="gb",
                           addr_space="Shared" if use_collective else "Local")
            if use_collective:
                nc.gpsimd.collective_compute(
                    "AllGather",
                    mybir.AluOpType.bypass,
                    replica_groups=[list(range(NCORES))],
                    ins=[bounce.opt()],
                    outs=[gb.opt()],
                )
            else:
                # timing-model stand-in: move the same bytes with plain DMAs
                for r in range(NCORES):
                    nc.sync.dma_start(gb[r * BB:(r + 1) * BB], bounce[:])

            # ---------- work that fills the gather bubble ----------
            vto_t = []
            for ec in range(2):
                t = workp.tile([128, NS], f32, name=f"vto{ec}", tag="vto")
                for mc in range(4):
                    tp = psum.tile([128, 128], f32, name="tp", tag="ps2")
                    nc.tensor.transpose(
                        tp[:], vc_sb[mc][:, ec * 128:(ec + 1) * 128], idn_sb[:])
                    nc.vector.tensor_copy(t[:, mc * 128:(mc + 1) * 128], tp[:])
                vto_t.append(t)
            qk_t = kto_t
            for ec in range(2):
                nc.vector.tensor_mul(qk_t[ec][:], qt_t[ec][:], kto_t[ec][:])
            edb = res.tile([128, NS], f32, name="edb", tag="edb")
            rcb = res.tile([128, NS], f32, name="rcb", tag="rcb")
            dnb = res.tile([128, NS], f32, name="dnb", tag="dnb")
            ed_h = [edb[h * 32:h * 32 + 1, :] for h in range(H)]
            rc_h = [rcb[h * 32:h * 32 + 1, :] for h in range(H)]
            dn0_h = [dnb[h * 32:h * 32 + 1, :] for h in range(H)]
            for h in range(H):
                ec, hp = h // 2, h % 2
                ds_ps = psum.tile([1, NS], f32, name=f"dsps{h}", tag="ps2")
                nc.tensor.matmul(
                    ds_ps[:],
                    ones_sb[hp * 64:hp * 64 + 64, :],
                    qk_t[ec][hp * 64:hp * 64 + 64, :].bitcast(f32),
                    start=True, stop=True,
                )
                nc.scalar.activation(ed_h[h], ds_ps[:], Exp, scale=float(SCALE))

            # ---------- gathered loads, ordered for phase-4 consumption ----------
            kt_t = []
            gbr = rbc(gb).rearrange("(r b) -> r b", r=NCORES)

            def load_kt(ec, split=1):
                t = res.tile([128, N], fr, name=f"kt{ec}", tag=f"kt{ec}")
                rr_ = NCORES // split
                for s in range(split):
                    nc.sync.dma_start(
                        t[:, s * rr_ * NS:(s + 1) * rr_ * NS].rearrange(
                            "p (r q) -> p r q", r=rr_),
                        gbr[s * rr_:(s + 1) * rr_,
                            VB + ec * 128 * NS:VB + (ec + 1) * 128 * NS]
                        .rearrange("r (p q) -> p r q", p=128),
                    )
                kt_t.append(t)

            # interleave kT(ec0) and v loads rank-by-rank so head pair 0's
            # scores and ctx both stream from rank 0 as soon as it lands
            kt0 = res.tile([128, N], fr, name="kt0", tag="kt0")
            v_t = []
            for r in range(NCORES):
                nc.sync.dma_start(
                    kt0[:, r * NS:(r + 1) * NS].rearrange(
                        "p (u q) -> p u q", u=1),
                    gbr[r:r + 1, VB:VB + 128 * NS].rearrange(
                        "u (p q) -> p u q", p=128),
                )
                t = res.tile([128, 4 * EW], fr, name=f"vr{r}", tag=f"vr{r}")
                nc.sync.dma_start(
                    t.rearrange("p (a w) -> p a w", a=4),
                    gbr[r:r + 1, 0:VB].rearrange(
                        "u (a p w) -> p (u a) w", a=4, p=128),
                )
                for mc in range(4):
                    v_t.append(t[:, mc * EW:(mc + 1) * EW])
            kt_t.append(kt0)
            load_kt(1)

            # ---------- phase 4: attention, two heads per pass ----------
            ctxu_sb = []
            for ec in range(2):
                t = res.tile([128, NS], fr, name=f"ctxu{ec}", tag=f"ctxu{ec}")
                ctxu_sb.append(t)
            ctxn_sb = []
            for ec in range(2):
                h0, h1 = 2 * ec, 2 * ec + 1
                cp = [pctx.tile([HD + 1, NS], f32, name=f"ctxps{h}", tag="ctx")
                      for h in (h0, h1)]
                for c in range(CH):
                    sc_ps = psum.tile([128, 2 * NS], f32, name="scps", tag="ps2")
                    # row-tiled pair: head h0 on PE rows 0-63, h1 on 64-127
                    nc.tensor.matmul(
                        sc_ps[:, 0:NS],
                        kt_t[ec][0:64, c * 128:(c + 1) * 128],
                        qt_t[ec][0:64, :],
                        start=True, stop=True,
                    )
                    nc.tensor.matmul(
                        sc_ps[:, NS:2 * NS],
                        kt_t[ec][64:128, c * 128:(c + 1) * 128],
                        qt_t[ec][64:128, :],
                        start=True, stop=True,
                    )
                    ex_t = expp.tile([128, 2 * NS], fr, name="ex", tag="ex")
                    nc.scalar.activation(ex_t[:], sc_ps[:], Exp,
                                         scale=float(SCALE))
                    for hq in range(2):
                        nc.tensor.matmul(
                            cp[hq][:],
                            v_t[c][:, (h0 + hq) * (HD + 1):
                                   (h0 + hq + 1) * (HD + 1)],
                            ex_t[:, hq * NS:(hq + 1) * NS],
                            start=(c == 0), stop=(c == CH - 1),
                        )
                for hq in range(2):
                    h = h0 + hq
                    nc.vector.tensor_copy(
                        ctxu_sb[ec][hq * 64:hq * 64 + 64, :], cp[hq][0:HD, :])
                    nc.vector.tensor_copy(dn0_h[h], cp[hq][HD:HD + 1, :])
                    nc.vector.tensor_sub(dn0_h[h], dn0_h[h], ed_h[h])
                    nc.vector.reciprocal(rc_h[h], dn0_h[h])
                # broadcast e_diag / recip rows over the head blocks, normalize
                eb_ps = pctx.tile([128, NS], f32, name="ebps", tag="ctx")
                rb_ps = pctx.tile([128, NS], f32, name="rbps", tag="ctx")
                for hq in range(2):
                    hh = h0 + hq
                    nc.tensor.matmul(eb_ps[hq * 64:hq * 64 + 64, :],
                                     ones2[hh * 32:hh * 32 + 1, :],
                                     ed_h[hh], start=True, stop=True,
                                     tile_position=(hh * 32, hq * 64))
                    nc.tensor.matmul(rb_ps[hq * 64:hq * 64 + 64, :],
                                     ones2[hh * 32:hh * 32 + 1, :],
                                     rc_h[hh], start=True, stop=True,
                                     tile_position=(hh * 32, hq * 64))
                nc.vector.tensor_mul(vto_t[ec][:], vto_t[ec][:], eb_ps[:])
                nc.vector.tensor_sub(ctxu_sb[ec][:], ctxu_sb[ec][:],
                                     vto_t[ec][:])
                nc.vector.tensor_mul(ctxu_sb[ec][:], ctxu_sb[ec][:], rb_ps[:])
                ctxn_sb.append(ctxu_sb[ec])

            # ---------- phase 6: out = ctx @ Wo.T (transposed) ----------
            oto_sb = []
            for ec in range(2):
                ps = psum.tile([128, NS], f32, name="pso", tag="ps2")
                for e in range(2):
                    nc.tensor.matmul(
                        ps[:],
                        wot_t[e][:, ec * 128:(ec + 1) * 128],
                        ctxn_sb[e][:],
                        start=(e == 0), stop=(e == 1),
                    )
                t = res.tile([128, NS], fr, name=f"oto{ec}", tag=f"kto{ec}")
                nc.vector.tensor_copy(t[:], ps[:])
                oto_sb.append(t)

            # ---------- phase 7: pred, sigmoid, multiply by A ----------
            # two pc chunks per iteration: one wdt DMA in, one result DMA out
            SKEW = 4
            wdc_q = []

            def issue_wdc(p2):
                if p2 % 2 == 0:
                    t = strm.tile([128, 512], fr, name="wdc", tag="wdc", bufs=2)
                else:
                    t = res.tile([128, 512], fr, name="wdc",
                                 tag=f"xq{(p2 // 2) % 2}")
                nc.sync.dma_start(
                    t.rearrange("p (e a q) -> p e a q", e=2, a=2),
                    rbc(wdt_d.ap()[:, p2 * 256:(p2 + 1) * 256]).rearrange(
                        "(e p) (a q) -> p e a q", e=2, a=2),
                )
                wdc_q.append(t.rearrange("p (e a q) -> p e a q", e=2, a=2))

            for p2 in range(SKEW):
                issue_wdc(p2)
            for p2 in range(CH // 2):
                if p2 + SKEW < CH // 2:
                    issue_wdc(p2 + SKEW)
                wdc = wdc_q[p2]
                ot_t = workp.tile([128, 2 * NS], f32, name="ot", tag="ot",
                                  bufs=3)
                for a in range(2):
                    pc = 2 * p2 + a
                    ps = psum.tile([128, NS], f32, name="psp", tag="ps2")
                    for e in range(2):
                        nc.tensor.matmul(
                            ps[:],
                            wdc[:, e, a, :],
                            oto_sb[e][:],
                            start=(e == 0), stop=(e == 1),
                        )
                    sg_t = res.tile([128, NS], f32, name="sg",
                                    tag=f"qt{(2 * p2 + a) % 2}")
                    nc.scalar.activation(sg_t[:], ps[:], Sig,
                                         bias=bd_sb[:, pc:pc + 1], scale=1.0)
                    j4, aa = pc // 4, pc % 4
                    nc.vector.tensor_mul(
                        ot_t[:, a * NS:(a + 1) * NS],
                        at_t[j4].bitcast(f32)[:, aa * NS:(aa + 1) * NS],
                        sg_t[:])
                nc.sync.dma_start(
                    out[p2 * 256:(p2 + 1) * 256, :].rearrange(
                        "(a p) q -> p a q", a=2),
                    ot_t.rearrange("p (a q) -> p a q", a=2))

    nc.compile()
    return nc


def _get_nc():
    if MM_MODE not in _cache:
        _cache[MM_MODE] = _build(MM_MODE)
    return _cache[MM_MODE]


def _prep_inputs(A, X, Wq, Wk, Wv, Wo, Wd, bd):
    """Host-side staging: transposes + per-core shards."""
    A = np.asarray(A, np.float32)
    X = np.asarray(X, np.float32)
    AT = np.ascontiguousarray(A.T)              # [N, N]; AT[:, r*NS:(r+1)*NS]
    XT = np.ascontiguousarray(X.T)              # [F, N]
    wqt = np.ascontiguousarray(np.asarray(Wq, np.float32).T)
    wkt = np.ascontiguousarray(np.asarray(Wk, np.float32).T)
    wvt = np.ascontiguousarray(np.asarray(Wv, np.float32).T)
    wot = np.ascontiguousarray(np.asarray(Wo, np.float32).T)
    wdt = np.ascontiguousarray(np.asarray(Wd, np.float32).T)
    bd_r = np.ascontiguousarray(np.asarray(bd, np.float32).reshape(CH, 128))
    idn = np.eye(128, dtype=np.float32)

    in_maps = []
    for r in range(NCORES):
        sl = slice(r * NS, (r + 1) * NS)
        in_maps.append({
            "at": np.ascontiguousarray(AT[:, sl]),
            "xq": np.ascontiguousarray(XT[:, sl]),
            "wqt": wqt, "wkt": wkt, "wvt": wvt, "wot": wot, "wdt": wdt,
            "bd": bd_r, "idn": idn,
        })
    return in_maps


def kernel(A, X, Wq, Wk, Wv, Wo, Wd, bd, numheads):
    from concourse import bass_utils

    assert int(numheads) == H
    nc = _get_nc()
    in_maps = _prep_inputs(A, X, Wq, Wk, Wv, Wo, Wd, bd)
    res = bass_utils.run_bass_kernel_spmd(nc, in_maps,
                                          core_ids=list(range(NCORES)))
    out = np.empty((N, N), np.float32)
    for r in range(NCORES):
        out[r * NS:(r + 1) * NS, :] = res.results[r]["outt"].T
    return out

